# revision 25
# baseline (speedup 1.0000x reference)
"""Trainium2 Bass kernel for nn_Baseline_SelfGCN (gnn_message_passing).

Data-parallel over batch: 8 NeuronCores x 8 images each. bf16 on device
(inputs/weights cast on host; PSUM accumulation stays f32), which halves
HBM traffic and doubles PE/DVE throughput.

x_gcn is loaded pre-transposed to (hw, c) layout by the DMA xbar
(dma_start_transpose). The 8 transpose-loads are issued back-to-back:
the Tile scheduler serializes transitions between xbar-transpose DMAs
and ordinary DMAs with completion waits, so batching them avoids ~2us
of dead time per transfer. All other DMAs are ordinary copies issued on
the SP queue in consumption order (streams -> W1 -> consts -> W2 ->
x_global), with output writebacks on the Act queue; W1/W2 stream through
4-deep conveyors whose ring reuse naturally paces the prefetch.

Per core:
  - segment raw sums for all 8 images accumulate into one PSUM block
    via a zero-padded block one-hot; counts + part-drop logic run in
    72-row space with a host-built block lower-triangular matrix
  - 2-layer GCN (x@W -> blockdiag(adjT)@s -> BN -> relu), both branches
    sharing the layer-1 raw matmul; 1/count and self-mask row scales
    fold into the post-layer-1 PSUM drain; (72,2048)->lhsT flips are PE
    transposes
  - mean over parts + BN(gn) -> bnfeat outputs; x2 concat written bf16
  - GAP of x_global (DVE reduce, streamed last) + BN(gb)

Host side: shard/layout/dtype staging only (bf16 casts, BN param folds,
block-diag adj^T assembly, mask downsample/permute); all reductions and
matmuls run on device. Output is written bf16 and upcast on host.

Self-contained: hardcodes shapes; host side only shards/gathers.
"""

import numpy as np
import ml_dtypes

import concourse.bass as bass
import concourse.mybir as mybir
import concourse.tile as tile
from concourse.masks import make_identity

F32 = mybir.dt.float32
BF = mybir.dt.bfloat16
I32 = mybir.dt.int32
AF = mybir.ActivationFunctionType
OP = mybir.AluOpType

BL = 8          # images per core
C = 2048
HW = 256        # Hf*Wf
NP = 9          # graph nodes (parts 1..9)
R = BL * NP     # 72 rows = (image, part)
EPS = 1e-5
NCH = 4         # 2048 / 512 N-chunks
KT = 16         # 2048 / 128 K-tiles
OUTW = 3 * C + 2 * NP * C  # 43008
NPBF = ml_dtypes.bfloat16


def legalize_waits(nc, max_waits=1):
    """Split multi-wait instructions: this walrus build allows only one
    embedded sync-wait per instruction; hoist extras onto standalone
    InstEventSemaphore waits on the same engine."""
    cnt = 0
    for fn in nc.m.functions:
        for blk in fn.blocks:
            out = []
            for inst in blk.instructions:
                si = inst.sync_info
                if si is not None and si.on_wait and len(si.on_wait) > max_waits:
                    waits = list(si.on_wait)
                    for w in waits[:-max_waits]:
                        cnt += 1
                        wi = mybir.InstEventSemaphore(
                            name=f"wsplit{cnt}_{inst.name}", ins=[], outs=[],
                            sync_info=mybir.SyncInfo(on_wait=[w], on_update=[]))
                        wi.engine = inst.engine
                        nc.register_instruction(wi)
                        out.append(wi)
                    si.on_wait = waits[-max_waits:]
                    inst.sync_info = si
                out.append(inst)
            blk.instructions = out
    return cnt


def build_bass():
    nc = bass.Bass()

    xg_p = nc.declare_dram_parameter("xg", [BL, C, HW], BF, isOutput=False)
    xc_p = nc.declare_dram_parameter("xc", [BL, C, HW], BF, isOutput=False)
    mk_p = nc.declare_dram_parameter("mkp", [128, 2, BL], I32, isOutput=False)
    bdl_p = nc.declare_dram_parameter("bdl", [R, 2, R], BF, isOutput=False)
    w1_p = nc.declare_dram_parameter("W1", [C, C], BF, isOutput=False)
    w2_p = nc.declare_dram_parameter("W2", [C, C], BF, isOutput=False)
    rep_p = nc.declare_dram_parameter("reps", [R, 4, C], BF, isOutput=False)
    sgb_p = nc.declare_dram_parameter("sgbt", [BL, 2, C], BF, isOutput=False)
    sgn_p = nc.declare_dram_parameter("sgnt", [BL, 2, C], BF, isOutput=False)
    out_p = nc.declare_dram_parameter("out", [BL, OUTW], BF, isOutput=True)

    with tile.TileContext(nc) as tc:
        with (
            tc.tile_pool(name="consts", bufs=1) as cs,
            tc.tile_pool(name="ps", bufs=8, space="PSUM") as ps,
        ):
            # mask load rides the SWDGE lanes, first so its lane barriers
            # land during DMA pipe-fill
            mki = cs.tile([128, 2, BL], I32)
            nc.gpsimd.dma_start(out=mki[:], in_=mk_p[:, :, :])

            # ---------------- constants (engine-built) ----------------
            ident = cs.tile([128, 128], BF)
            make_identity(nc, ident[:])

            iota_i = cs.tile([128, NP], I32)
            nc.gpsimd.iota(iota_i[:], pattern=[[1, NP]], base=1,
                           channel_multiplier=0)
            iota_f = cs.tile([128, NP], BF)
            nc.gpsimd.tensor_copy(out=iota_f[:], in_=iota_i[:])

            ones_col = cs.tile([128, 1], BF)
            nc.gpsimd.memset(ones_col[:], 1.0)

            # block "mean over parts" matrix (72, 8): 1/9 on image blocks
            onesblk = cs.tile([R, BL], BF)
            nc.gpsimd.memset(onesblk[:], 1.0 / NP)
            nc.gpsimd.affine_select(
                out=onesblk[:], in_=onesblk[:], compare_op=OP.is_ge, fill=0.0,
                base=0, pattern=[[-NP, BL]], channel_multiplier=1)
            nc.gpsimd.affine_select(
                out=onesblk[:], in_=onesblk[:], compare_op=OP.is_ge, fill=0.0,
                base=NP - 1, pattern=[[NP, BL]], channel_multiplier=-1)

            bdl = cs.tile([R, 2, R], BF)
            BDr = bdl[:, 0, :]
            L72 = bdl[:, 1, :]
            rep4 = cs.tile([R, 4, C], BF)
            s1r, t1r = rep4[:, 0, :], rep4[:, 1, :]
            s2r, t2r = rep4[:, 2, :], rep4[:, 3, :]
            sgbt = cs.tile([BL, 2, C], BF)
            sgb, tgb = sgbt[:, 0, :], sgbt[:, 1, :]
            sgnt = cs.tile([BL, 2, C], BF)
            sgn, tgn = sgnt[:, 0, :], sgnt[:, 1, :]

            # image-selector lhsT for the PE GAP: I8[:, b, m] = (m == b)
            I8 = cs.tile([128, BL, BL], BF)
            nc.gpsimd.memset(I8[:], 0.0)
            for b in range(BL):
                nc.gpsimd.memset(I8[:, b, b:b + 1], 1.0)

            # ---------------- mask -> padded block onehot ----------------
            mrf = cs.tile([128, 2, BL], F32)
            nc.vector.tensor_copy(out=mrf[:], in_=mki[:])
            ohz = cs.tile([128, 2, BL, R], BF)
            nc.gpsimd.memset(ohz[:], 0.0)
            for h in range(2):
                for b in range(BL):
                    nc.vector.tensor_scalar(
                        out=ohz[:, h, b, NP * b:NP * (b + 1)], in0=iota_f[:],
                        scalar1=mrf[:, h, b:b + 1], scalar2=None,
                        op0=OP.is_equal)

            mfT = cs.tile([128, KT, R], BF)     # layer-1 lhsT (raw sums^T)

            with (
                tc.tile_pool(name="stream", bufs=4) as stream,
                tc.tile_pool(name="gstream", bufs=2) as gstream,
                tc.tile_pool(name="wp", bufs=4) as wp,
                tc.tile_pool(name="wp2", bufs=4) as wp2,
                tc.tile_pool(name="mm", bufs=1) as mm,
            ):
                # ---------- x_gcn xbar-transpose loads, batched ----------
                xcT = []
                for b in range(BL):
                    t = stream.tile([128, 2, C], BF, tag="xcT", name=f"xcT{b}")
                    nc.sync.dma_start_transpose(t[:], xc_p[b])
                    xcT.append(t)

                # ---------- ordinary DMAs, in consumption order ----------
                nc.sync.dma_start(out=bdl[:], in_=bdl_p[:, :, :])
                nc.sync.dma_start(out=rep4[:], in_=rep_p[:, :, :])
                nc.sync.dma_start(out=sgnt[:], in_=sgn_p[:, :, :])
                nc.sync.dma_start(out=sgbt[:], in_=sgb_p[:, :, :])
                w1t = [wp.tile([128, 2, C], BF, tag="w", name=f"w1_{kp}")
                       for kp in range(8)]
                for kp in range(8):
                    nc.sync.dma_start(
                        out=w1t[kp][:],
                        in_=w1_p[256 * kp:256 * (kp + 1), :].rearrange(
                            "(t p) c -> p t c", p=128))
                # W2 conveyor fully ahead of x_global: the GAP/global
                # branch has the shortest post-DMA tail, so it drains last
                w2t = [wp2.tile([128, 2, C], BF, tag="w2", name=f"w2_{kp}")
                       for kp in range(8)]
                for kp in range(8):
                    nc.sync.dma_start(
                        out=w2t[kp][:],
                        in_=w2_p[256 * kp:256 * (kp + 1), :].rearrange(
                            "(t p) c -> p t c", p=128))
                xgT = []
                for b in range(BL):
                    t = gstream.tile([128, 2, C], BF, tag="xg",
                                     name=f"xgT{b}")
                    nc.sync.dma_start_transpose(t[:], xg_p[b])
                    xgT.append(t)

                # ---------- pooling + counts (PE, chases the stream) ------
                pc72 = ps.tile([R, 1], F32, tag="ps", name="pc72")
                praw = [ps.tile([R, 512], F32, tag="ps", name=f"praw{n}")
                        for n in range(NCH)]
                for b in range(BL):
                    for h in range(2):
                        st = (b == 0 and h == 0)
                        sp = (b == BL - 1 and h == 1)
                        lhsT = ohz[:, h, b, :]
                        nc.tensor.matmul(pc72[:], lhsT, ones_col[:],
                                         start=st, stop=sp)
                        for n in range(NCH):
                            nc.tensor.matmul(
                                praw[n][:], lhsT,
                                xcT[b][:, h, 512 * n:512 * (n + 1)],
                                start=st, stop=sp)

                # ---------- counts -> row scales (72-row space) ----------
                rec72 = cs.tile([R, 1], F32)
                nc.vector.tensor_scalar_add(rec72[:], pc72[:], 1e-8)
                nc.vector.reciprocal(out=rec72[:], in_=rec72[:])
                pres72 = cs.tile([R, 1], BF)
                nc.vector.tensor_scalar(out=pres72[:], in0=pc72[:],
                                        scalar1=0.0, scalar2=None,
                                        op0=OP.is_gt)
                ppre72 = ps.tile([R, 1], F32, tag="ps", name="ppre72")
                nc.tensor.matmul(ppre72[:], L72[:], pres72[:],
                                 start=True, stop=True)
                srec72 = cs.tile([R, 1], F32)
                nc.vector.tensor_scalar(out=srec72[:], in0=ppre72[:],
                                        scalar1=0.0, scalar2=None,
                                        op0=OP.is_equal)
                nc.vector.tensor_tensor(srec72[:], srec72[:], pres72[:],
                                        OP.mult)
                # srec = (1 - first_present) * rec
                nc.vector.tensor_scalar(out=srec72[:], in0=srec72[:],
                                        scalar1=-1.0, scalar2=1.0,
                                        op0=OP.mult, op1=OP.add)
                nc.vector.tensor_tensor(srec72[:], srec72[:], rec72[:],
                                        OP.mult)

                # ---------- (72, 2048) -> lhsT via PE transposes ----------
                def pe_flip(src, dst, br):
                    """src (72, 2048) bf16 sbuf -> dst (128, 16, 72) bf16."""
                    for q in range(4):
                        ptr = ps.tile([128, 4, R], BF, tag="ps",
                                      name=f"ptr{br}{q}")
                        for t in range(4):
                            kt = 4 * q + t
                            nc.tensor.transpose(
                                ptr[:, t, :], src[:, 128 * kt:128 * (kt + 1)],
                                ident[0:R, 0:R])
                        nc.scalar.activation(
                            out=dst[:, 4 * q:4 * (q + 1), :],
                            in_=ptr[:], func=AF.Copy)

                rawS = mm.tile([R, C], BF, tag="rawS")
                for n in range(NCH):
                    if n % 2 == 0:
                        nc.scalar.activation(
                            out=rawS[:, 512 * n:512 * (n + 1)],
                            in_=praw[n][:], func=AF.Copy)
                    else:
                        nc.vector.tensor_copy(
                            out=rawS[:, 512 * n:512 * (n + 1)],
                            in_=praw[n][:])
                pe_flip(rawS, mfT, "mf")

                # ---------- GCN layer 1 ----------
                psl1 = [ps.tile([R, 512], F32, tag="ps", name=f"psl1_{n}")
                        for n in range(NCH)]
                for kp in range(8):
                    w = w1t[kp]
                    for t in range(2):
                        kt = 2 * kp + t
                        for n in range(NCH):
                            nc.tensor.matmul(
                                psl1[n][:], mfT[:, kt, :],
                                w[:, t, 512 * n:512 * (n + 1)],
                                start=(kt == 0), stop=(kt == KT - 1))

                s_all = mm.tile([R, C], BF, tag="rawS")
                s_self = mm.tile([R, C], BF, tag="sb")
                for n in range(NCH):
                    sl = slice(512 * n, 512 * (n + 1))
                    nc.vector.tensor_scalar(
                        out=s_all[:, sl], in0=psl1[n][:],
                        scalar1=rec72[:, 0:1], scalar2=None, op0=OP.mult)
                    nc.vector.tensor_scalar(
                        out=s_self[:, sl], in0=psl1[n][:],
                        scalar1=srec72[:, 0:1], scalar2=None, op0=OP.mult)

                # ---------- bmm + BN + relu (layer l, both branches) ------
                def bmm_bn_relu(s_in, sr, tr, xo, br):
                    for n in range(NCH):
                        sl = slice(512 * n, 512 * (n + 1))
                        po = ps.tile([R, 512], F32, tag="ps",
                                     name=f"po{br}{n}")
                        nc.tensor.matmul(po[:], BDr[:], s_in[:, sl],
                                         start=True, stop=True)
                        nc.vector.tensor_tensor(xo[:, sl], po[:],
                                                sr[:, sl], OP.mult)
                        nc.gpsimd.tensor_tensor(xo[:, sl], xo[:, sl],
                                                tr[:, sl], OP.add)
                        nc.scalar.activation(out=xo[:, sl], in_=xo[:, sl],
                                             func=AF.Relu)

                x1m = mm.tile([R, C], BF, tag="x1m")
                x1s = mm.tile([R, C], BF, tag="x1s")
                bmm_bn_relu(s_all, s1r, t1r, x1m, "m1")
                bmm_bn_relu(s_self, s1r, t1r, x1s, "s1")

                x1T = {}
                for br, x1 in (("m", x1m), ("s", x1s)):
                    xt1 = mm.tile([128, KT, R], BF, tag=f"x1T{br}")
                    pe_flip(x1, xt1, br)
                    x1T[br] = xt1

                # ---------- GCN layer 2 ----------
                psl2 = {br: [ps.tile([R, 512], F32, tag="ps",
                                     name=f"psl2_{br}_{n}")
                             for n in range(NCH)] for br in ("m", "s")}
                for kp in range(8):
                    w = w2t[kp]
                    for t in range(2):
                        kt = 2 * kp + t
                        for br in ("m", "s"):
                            for n in range(NCH):
                                nc.tensor.matmul(
                                    psl2[br][n][:], x1T[br][:, kt, :],
                                    w[:, t, 512 * n:512 * (n + 1)],
                                    start=(kt == 0), stop=(kt == KT - 1))

                s2m = mm.tile([R, C], BF, tag="rawS")
                s2s = mm.tile([R, C], BF, tag="sb")
                for br, s2 in (("m", s2m), ("s", s2s)):
                    for n in range(NCH):
                        sl = slice(512 * n, 512 * (n + 1))
                        nc.scalar.activation(out=s2[:, sl],
                                             in_=psl2[br][n][:],
                                             func=AF.Copy)

                # ---------- layer-2 bmm + BN + relu; outputs ----------
                cat_off = {"m": 3 * C, "s": 3 * C + NP * C}
                bnf_off = {"m": C, "s": 2 * C}
                for br, s2 in (("m", s2m), ("s", s2s)):
                    x2 = mm.tile([R, C], BF, tag=f"x1{br}")
                    bmm_bn_relu(s2, s2r, t2r, x2, f"{br}2")
                    off = cat_off[br]
                    nc.scalar.dma_start(
                        out=out_p[:, off:off + NP * C].rearrange(
                            "b (p d) -> b p d", d=C),
                        in_=x2[:])
                    # mean over parts + BN(gn)
                    bnf = mm.tile([BL, C], BF, tag="bnf")
                    for n in range(NCH):
                        sl = slice(512 * n, 512 * (n + 1))
                        pf = ps.tile([BL, 512], F32, tag="ps",
                                     name=f"pf{br}{n}")
                        nc.tensor.matmul(pf[:], onesblk[:], x2[:, sl],
                                         start=True, stop=True)
                        nc.vector.tensor_tensor(bnf[:, sl], pf[:],
                                                sgn[:, sl], OP.mult)
                        nc.gpsimd.tensor_tensor(bnf[:, sl], bnf[:, sl],
                                                tgn[:, sl], OP.add)
                    boff = bnf_off[br]
                    nc.scalar.dma_start(out=out_p[:, boff:boff + C],
                                        in_=bnf[:])

                # ---------- GAP on PE (drains last) ----------
                # selector-matmul: out row b accumulates image b's spatial
                # sum; bnfeat_global lands directly in (b, c) layout
                pgap = [ps.tile([BL, 512], F32, tag="ps", name=f"pgap{n}")
                        for n in range(NCH)]
                for b in range(BL):
                    for h in range(2):
                        st = (b == 0 and h == 0)
                        sp = (b == BL - 1 and h == 1)
                        for n in range(NCH):
                            nc.tensor.matmul(
                                pgap[n][:], I8[:, b, :],
                                xgT[b][:, h, 512 * n:512 * (n + 1)],
                                start=st, stop=sp)
                Gn = cs.tile([BL, C], BF)
                for n in range(NCH):
                    sl = slice(512 * n, 512 * (n + 1))
                    nc.vector.tensor_tensor(Gn[:, sl], pgap[n][:],
                                            sgb[:, sl], OP.mult)
                    nc.gpsimd.tensor_tensor(Gn[:, sl], Gn[:, sl],
                                            tgb[:, sl], OP.add)
                nc.scalar.dma_start(out=out_p[:, 0:C], in_=Gn[:])

    legalize_waits(nc)
    return nc


_CACHE = {}


def _fold_bn(g, be, rm, rv, blin):
    s = g / np.sqrt(rv + EPS)
    t = (blin - rm) * s + be
    return s, t


def kernel(_run_kwargs=None, **inputs):
    run_kwargs = _run_kwargs or {}
    if "nc" not in _CACHE:
        _CACHE["nc"] = build_bass()
    nc = _CACHE["nc"]

    B = inputs["x_global"].shape[0]
    n_cores = 8
    bl = B // n_cores

    f = {k: np.asarray(inputs[k], np.float32) for k in inputs if k != "mask"}

    # BN folds (parameter preprocessing, replicated per core)
    s1, t1 = _fold_bn(f["g1"].reshape(NP, C), f["be1"].reshape(NP, C),
                      f["rm1"].reshape(NP, C), f["rv1"].reshape(NP, C),
                      f["b1"][None, :])
    s2, t2 = _fold_bn(f["g2"].reshape(NP, C), f["be2"].reshape(NP, C),
                      f["rm2"].reshape(NP, C), f["rv2"].reshape(NP, C),
                      f["b2"][None, :])
    reps = np.ascontiguousarray(np.stack(
        [np.tile(s1, (BL, 1)), np.tile(t1, (BL, 1)),
         np.tile(s2, (BL, 1)), np.tile(t2, (BL, 1))], axis=1)).astype(NPBF)
    sgb_ = f["gb_g"] / np.sqrt(f["gb_rv"] + EPS)
    tgb_ = f["gb_b"] - f["gb_rm"] * sgb_
    sgbt = np.ascontiguousarray(np.stack(
        [np.tile((sgb_ / HW)[None, :], (BL, 1)),
         np.tile(tgb_[None, :], (BL, 1))], axis=1)).astype(NPBF)
    sgn_ = f["gn_g"] / np.sqrt(f["gn_rv"] + EPS)
    tgn_ = f["gn_b"] - f["gn_rm"] * sgn_
    sgnt = np.ascontiguousarray(np.stack(
        [np.tile(sgn_[None, :], (BL, 1)),
         np.tile(tgn_[None, :], (BL, 1))], axis=1)).astype(NPBF)
    w1 = f["W1"].astype(NPBF)
    w2 = f["W2"].astype(NPBF)

    # block-diag strictly-lower-triangular (constant)
    l72 = np.zeros((R, R), np.float32)
    blk = np.tril(np.ones((NP, NP), np.float32), -1).T  # [q,r]=1 iff q<r
    for b in range(BL):
        l72[NP * b:NP * (b + 1), NP * b:NP * (b + 1)] = blk
    l72 = l72.astype(NPBF)

    mask_ds = np.asarray(inputs["mask"])[:, 0, ::16, ::16]  # (B, 16, 16)

    in_maps = []
    for c in range(n_cores):
        sl = slice(c * bl, (c + 1) * bl)
        # mask (bl,16,16) -> [p, h, b] with hw = 128*h + p
        md = mask_ds[sl].reshape(bl, 256).T.reshape(2, 128, bl)
        adj = f["adj"][sl]
        bd = np.zeros((R, R), np.float32)
        for b in range(bl):
            bd[NP * b:NP * (b + 1), NP * b:NP * (b + 1)] = adj[b].T
        bdl = np.ascontiguousarray(np.stack(
            [bd.astype(NPBF), l72], axis=1))
        m = {
            "xg": np.ascontiguousarray(f["x_global"][sl]).reshape(
                bl, C, HW).astype(NPBF),
            "xc": np.ascontiguousarray(f["x_gcn"][sl]).reshape(
                bl, C, HW).astype(NPBF),
            "mkp": np.ascontiguousarray(md.transpose(1, 0, 2)).astype(np.int32),
            "bdl": bdl,
            "W1": w1, "W2": w2,
            "reps": reps,
            "sgbt": sgbt, "sgnt": sgnt,
        }
        in_maps.append(m)

    from concourse.bass_utils import run_bass_kernel_spmd
    res = run_bass_kernel_spmd(nc, in_maps, list(range(n_cores)), **run_kwargs)
    out = np.concatenate(
        [np.asarray(res.results[c]["out"]).astype(np.float32)
         for c in range(n_cores)], axis=0)
    _CACHE["last_results"] = res
    return out


# revision 26
# speedup vs baseline: 1.1514x; 1.1514x over previous
"""Trainium2 Bass kernel for nn_Baseline_SelfGCN (gnn_message_passing).

Data-parallel over batch: 8 NeuronCores x 8 images each. bf16 on device
(inputs/weights cast on host; PSUM accumulation stays f32), which halves
HBM traffic and doubles PE/DVE throughput.

x_gcn is loaded pre-transposed to (hw, c) layout by the DMA xbar
(dma_start_transpose). The 8 transpose-loads are issued back-to-back:
the Tile scheduler serializes transitions between xbar-transpose DMAs
and ordinary DMAs with completion waits, so batching them avoids ~2us
of dead time per transfer. All other DMAs are ordinary copies issued on
the SP queue in consumption order (streams -> W1 -> consts -> W2 ->
x_global), with output writebacks on the Act queue; W1/W2 stream through
4-deep conveyors whose ring reuse naturally paces the prefetch.

Per core:
  - segment raw sums for all 8 images accumulate into one PSUM block
    via a zero-padded block one-hot; counts + part-drop logic run in
    72-row space with a host-built block lower-triangular matrix
  - 2-layer GCN (x@W -> blockdiag(adjT)@s -> BN -> relu), both branches
    sharing the layer-1 raw matmul; 1/count and self-mask row scales
    fold into the post-layer-1 PSUM drain; (72,2048)->lhsT flips are PE
    transposes
  - mean over parts + BN(gn) -> bnfeat outputs; x2 concat written bf16
  - GAP of x_global (DVE reduce, streamed last) + BN(gb)

Host side: shard/layout/dtype staging only (bf16 casts, BN param folds,
block-diag adj^T assembly, mask downsample/permute); all reductions and
matmuls run on device. Output is written bf16 and upcast on host.

Self-contained: hardcodes shapes; host side only shards/gathers.
"""

import numpy as np
import ml_dtypes

import concourse.bass as bass
import concourse.mybir as mybir
import concourse.tile as tile
from concourse.masks import make_identity

F32 = mybir.dt.float32
BF = mybir.dt.bfloat16
I32 = mybir.dt.int32
AF = mybir.ActivationFunctionType
OP = mybir.AluOpType

BL = 8          # images per core
C = 2048
HW = 256        # Hf*Wf
NP = 9          # graph nodes (parts 1..9)
R = BL * NP     # 72 rows = (image, part)
EPS = 1e-5
NCH = 4         # 2048 / 512 N-chunks
KT = 16         # 2048 / 128 K-tiles
OUTW = 3 * C + 2 * NP * C  # 43008
NPBF = ml_dtypes.bfloat16


def legalize_waits(nc, max_waits=1):
    """Split multi-wait instructions: this walrus build allows only one
    embedded sync-wait per instruction; hoist extras onto standalone
    InstEventSemaphore waits on the same engine."""
    cnt = 0
    for fn in nc.m.functions:
        for blk in fn.blocks:
            out = []
            for inst in blk.instructions:
                si = inst.sync_info
                if si is not None and si.on_wait and len(si.on_wait) > max_waits:
                    waits = list(si.on_wait)
                    for w in waits[:-max_waits]:
                        cnt += 1
                        wi = mybir.InstEventSemaphore(
                            name=f"wsplit{cnt}_{inst.name}", ins=[], outs=[],
                            sync_info=mybir.SyncInfo(on_wait=[w], on_update=[]))
                        wi.engine = inst.engine
                        nc.register_instruction(wi)
                        out.append(wi)
                    si.on_wait = waits[-max_waits:]
                    inst.sync_info = si
                out.append(inst)
            blk.instructions = out
    return cnt


def build_bass():
    nc = bass.Bass()

    xg_p = nc.declare_dram_parameter("xg", [BL, C, HW], BF, isOutput=False)
    xc_p = nc.declare_dram_parameter("xc", [BL, C, HW], BF, isOutput=False)
    mk_p = nc.declare_dram_parameter("mkp", [128, 2, BL], I32, isOutput=False)
    bdl_p = nc.declare_dram_parameter("bdl", [R, 2, R], BF, isOutput=False)
    w1_p = nc.declare_dram_parameter("W1", [C, C], BF, isOutput=False)
    w2_p = nc.declare_dram_parameter("W2", [C, C], BF, isOutput=False)
    rep_p = nc.declare_dram_parameter("reps", [R, 4, C], BF, isOutput=False)
    sgb_p = nc.declare_dram_parameter("sgbt", [BL, 2, C], BF, isOutput=False)
    sgn_p = nc.declare_dram_parameter("sgnt", [BL, 2, C], BF, isOutput=False)
    out_p = nc.declare_dram_parameter("out", [BL, OUTW], BF, isOutput=True)

    with tile.TileContext(nc) as tc:
        with (
            tc.tile_pool(name="consts", bufs=1) as cs,
            tc.tile_pool(name="ps", bufs=8, space="PSUM") as ps,
        ):
            # mask load rides the SWDGE lanes, first so its lane barriers
            # land during DMA pipe-fill
            mki = cs.tile([128, 2, BL], I32)
            nc.gpsimd.dma_start(out=mki[:], in_=mk_p[:, :, :])

            # ---------------- constants (engine-built) ----------------
            ident = cs.tile([128, 128], BF)
            make_identity(nc, ident[:])

            iota_i = cs.tile([128, NP], I32)
            nc.gpsimd.iota(iota_i[:], pattern=[[1, NP]], base=1,
                           channel_multiplier=0)
            iota_f = cs.tile([128, NP], BF)
            nc.gpsimd.tensor_copy(out=iota_f[:], in_=iota_i[:])

            ones_col = cs.tile([128, 1], BF)
            nc.gpsimd.memset(ones_col[:], 1.0)

            # block "mean over parts" matrix (72, 8): 1/9 on image blocks
            onesblk = cs.tile([R, BL], BF)
            nc.gpsimd.memset(onesblk[:], 1.0 / NP)
            nc.gpsimd.affine_select(
                out=onesblk[:], in_=onesblk[:], compare_op=OP.is_ge, fill=0.0,
                base=0, pattern=[[-NP, BL]], channel_multiplier=1)
            nc.gpsimd.affine_select(
                out=onesblk[:], in_=onesblk[:], compare_op=OP.is_ge, fill=0.0,
                base=NP - 1, pattern=[[NP, BL]], channel_multiplier=-1)

            bdl = cs.tile([R, 2, R], BF)
            BDr = bdl[:, 0, :]
            L72 = bdl[:, 1, :]
            rep4 = cs.tile([R, 4, C], BF)
            s1r, t1r = rep4[:, 0, :], rep4[:, 1, :]
            s2r, t2r = rep4[:, 2, :], rep4[:, 3, :]
            sgbt = cs.tile([BL, 2, C], BF)
            sgb, tgb = sgbt[:, 0, :], sgbt[:, 1, :]
            sgnt = cs.tile([BL, 2, C], BF)
            sgn, tgn = sgnt[:, 0, :], sgnt[:, 1, :]

            # image-selector lhsT for the PE GAP: I8[:, b, m] = (m == b)
            I8 = cs.tile([128, BL, BL], BF)
            nc.gpsimd.memset(I8[:], 0.0)
            for b in range(BL):
                nc.gpsimd.memset(I8[:, b, b:b + 1], 1.0)

            # ---------------- mask -> padded block onehot ----------------
            mrf = cs.tile([128, 2, BL], F32)
            nc.vector.tensor_copy(out=mrf[:], in_=mki[:])
            ohz = cs.tile([128, 2, BL, R], BF)
            nc.gpsimd.memset(ohz[:], 0.0)
            for h in range(2):
                for b in range(BL):
                    nc.vector.tensor_scalar(
                        out=ohz[:, h, b, NP * b:NP * (b + 1)], in0=iota_f[:],
                        scalar1=mrf[:, h, b:b + 1], scalar2=None,
                        op0=OP.is_equal)

            mfT = cs.tile([128, KT, R], BF)     # layer-1 lhsT (raw sums^T)

            with (
                tc.tile_pool(name="stream", bufs=4) as stream,
                tc.tile_pool(name="gstream", bufs=4) as gstream,
                tc.tile_pool(name="wp", bufs=4) as wp,
                tc.tile_pool(name="wp2", bufs=4) as wp2,
                tc.tile_pool(name="mm", bufs=1) as mm,
            ):
                # ---------- x_gcn xbar-transpose loads, batched ----------
                xcT = []
                for b in range(BL):
                    t = stream.tile([128, 2, C], BF, tag="xcT", name=f"xcT{b}")
                    nc.sync.dma_start_transpose(t[:], xc_p[b])
                    xcT.append(t)

                # ---------- ordinary DMAs, in consumption order ----------
                nc.sync.dma_start(out=bdl[:], in_=bdl_p[:, :, :])
                nc.sync.dma_start(out=rep4[:], in_=rep_p[:, :, :])
                nc.sync.dma_start(out=sgnt[:], in_=sgn_p[:, :, :])
                nc.sync.dma_start(out=sgbt[:], in_=sgb_p[:, :, :])
                w1t = [wp.tile([128, 2, C], BF, tag="w", name=f"w1_{kp}")
                       for kp in range(8)]
                for kp in range(8):
                    nc.sync.dma_start(
                        out=w1t[kp][:],
                        in_=w1_p[256 * kp:256 * (kp + 1), :].rearrange(
                            "(t p) c -> p t c", p=128))
                # W2 conveyor fully ahead of x_global: the GAP/global
                # branch has the shortest post-DMA tail, so it drains last
                w2t = [wp2.tile([128, 2, C], BF, tag="w2", name=f"w2_{kp}")
                       for kp in range(8)]
                for kp in range(8):
                    nc.sync.dma_start(
                        out=w2t[kp][:],
                        in_=w2_p[256 * kp:256 * (kp + 1), :].rearrange(
                            "(t p) c -> p t c", p=128))
                xgT = []
                for b in range(BL):
                    t = gstream.tile([128, 2, C], BF, tag="xg",
                                     name=f"xgT{b}")
                    nc.sync.dma_start_transpose(t[:], xg_p[b])
                    xgT.append(t)

                # ---------- pooling + counts (PE, chases the stream) ------
                pc72 = ps.tile([R, 1], F32, tag="ps", name="pc72")
                praw = [ps.tile([R, 512], F32, tag="ps", name=f"praw{n}")
                        for n in range(NCH)]
                for b in range(BL):
                    for h in range(2):
                        st = (b == 0 and h == 0)
                        sp = (b == BL - 1 and h == 1)
                        lhsT = ohz[:, h, b, :]
                        nc.tensor.matmul(pc72[:], lhsT, ones_col[:],
                                         start=st, stop=sp)
                        for n in range(NCH):
                            nc.tensor.matmul(
                                praw[n][:], lhsT,
                                xcT[b][:, h, 512 * n:512 * (n + 1)],
                                start=st, stop=sp)

                # ---------- counts -> row scales (72-row space) ----------
                rec72 = cs.tile([R, 1], F32)
                nc.vector.tensor_scalar_add(rec72[:], pc72[:], 1e-8)
                nc.vector.reciprocal(out=rec72[:], in_=rec72[:])
                pres72 = cs.tile([R, 1], BF)
                nc.vector.tensor_scalar(out=pres72[:], in0=pc72[:],
                                        scalar1=0.0, scalar2=None,
                                        op0=OP.is_gt)
                ppre72 = ps.tile([R, 1], F32, tag="ps", name="ppre72")
                nc.tensor.matmul(ppre72[:], L72[:], pres72[:],
                                 start=True, stop=True)
                srec72 = cs.tile([R, 1], F32)
                nc.vector.tensor_scalar(out=srec72[:], in0=ppre72[:],
                                        scalar1=0.0, scalar2=None,
                                        op0=OP.is_equal)
                nc.vector.tensor_tensor(srec72[:], srec72[:], pres72[:],
                                        OP.mult)
                # srec = (1 - first_present) * rec
                nc.vector.tensor_scalar(out=srec72[:], in0=srec72[:],
                                        scalar1=-1.0, scalar2=1.0,
                                        op0=OP.mult, op1=OP.add)
                nc.vector.tensor_tensor(srec72[:], srec72[:], rec72[:],
                                        OP.mult)

                # ---------- (72, 2048) -> lhsT via PE transposes ----------
                def pe_flip(src, dst, br):
                    """src (72, 2048) bf16 sbuf -> dst (128, 16, 72) bf16."""
                    for q in range(4):
                        ptr = ps.tile([128, 4, R], BF, tag="ps",
                                      name=f"ptr{br}{q}")
                        for t in range(4):
                            kt = 4 * q + t
                            nc.tensor.transpose(
                                ptr[:, t, :], src[:, 128 * kt:128 * (kt + 1)],
                                ident[0:R, 0:R])
                        nc.scalar.activation(
                            out=dst[:, 4 * q:4 * (q + 1), :],
                            in_=ptr[:], func=AF.Copy)

                rawS = mm.tile([R, C], BF, tag="rawS")
                for n in range(NCH):
                    if n % 2 == 0:
                        nc.scalar.activation(
                            out=rawS[:, 512 * n:512 * (n + 1)],
                            in_=praw[n][:], func=AF.Copy)
                    else:
                        nc.vector.tensor_copy(
                            out=rawS[:, 512 * n:512 * (n + 1)],
                            in_=praw[n][:])
                pe_flip(rawS, mfT, "mf")

                # ---------- GCN layer 1 ----------
                psl1 = [ps.tile([R, 512], F32, tag="ps", name=f"psl1_{n}")
                        for n in range(NCH)]
                for kp in range(8):
                    w = w1t[kp]
                    for t in range(2):
                        kt = 2 * kp + t
                        for n in range(NCH):
                            nc.tensor.matmul(
                                psl1[n][:], mfT[:, kt, :],
                                w[:, t, 512 * n:512 * (n + 1)],
                                start=(kt == 0), stop=(kt == KT - 1))

                s_all = mm.tile([R, C], BF, tag="rawS")
                s_self = mm.tile([R, C], BF, tag="sb")
                for n in range(NCH):
                    sl = slice(512 * n, 512 * (n + 1))
                    nc.vector.tensor_scalar(
                        out=s_all[:, sl], in0=psl1[n][:],
                        scalar1=rec72[:, 0:1], scalar2=None, op0=OP.mult)
                    nc.vector.tensor_scalar(
                        out=s_self[:, sl], in0=psl1[n][:],
                        scalar1=srec72[:, 0:1], scalar2=None, op0=OP.mult)

                # ---------- bmm + BN + relu (layer l, both branches) ------
                def bmm_bn_relu(s_in, sr, tr, xo, br):
                    for n in range(NCH):
                        sl = slice(512 * n, 512 * (n + 1))
                        po = ps.tile([R, 512], F32, tag="ps",
                                     name=f"po{br}{n}")
                        nc.tensor.matmul(po[:], BDr[:], s_in[:, sl],
                                         start=True, stop=True)
                        nc.vector.tensor_tensor(xo[:, sl], po[:],
                                                sr[:, sl], OP.mult)
                        nc.gpsimd.tensor_tensor(xo[:, sl], xo[:, sl],
                                                tr[:, sl], OP.add)
                        nc.scalar.activation(out=xo[:, sl], in_=xo[:, sl],
                                             func=AF.Relu)

                x1m = mm.tile([R, C], BF, tag="x1m")
                x1s = mm.tile([R, C], BF, tag="x1s")
                bmm_bn_relu(s_all, s1r, t1r, x1m, "m1")
                bmm_bn_relu(s_self, s1r, t1r, x1s, "s1")

                x1T = {}
                for br, x1 in (("m", x1m), ("s", x1s)):
                    xt1 = mm.tile([128, KT, R], BF, tag=f"x1T{br}")
                    pe_flip(x1, xt1, br)
                    x1T[br] = xt1

                # ---------- GCN layer 2 ----------
                psl2 = {br: [ps.tile([R, 512], F32, tag="ps",
                                     name=f"psl2_{br}_{n}")
                             for n in range(NCH)] for br in ("m", "s")}
                for kp in range(8):
                    w = w2t[kp]
                    for t in range(2):
                        kt = 2 * kp + t
                        for br in ("m", "s"):
                            for n in range(NCH):
                                nc.tensor.matmul(
                                    psl2[br][n][:], x1T[br][:, kt, :],
                                    w[:, t, 512 * n:512 * (n + 1)],
                                    start=(kt == 0), stop=(kt == KT - 1))

                # PE GAP selector-matmuls, interleaved with the tail so
                # xgT ring buffers free as transfers land
                pgap = [ps.tile([BL, 512], F32, tag="ps", name=f"pgap{n}")
                        for n in range(NCH)]

                def gap_mm(b):
                    for h in range(2):
                        st = (b == 0 and h == 0)
                        sp = (b == BL - 1 and h == 1)
                        for n in range(NCH):
                            nc.tensor.matmul(
                                pgap[n][:], I8[:, b, :],
                                xgT[b][:, h, 512 * n:512 * (n + 1)],
                                start=st, stop=sp)

                gap_mm(0)
                gap_mm(1)

                s2m = mm.tile([R, C], BF, tag="rawS")
                s2s = mm.tile([R, C], BF, tag="sb")
                for br, s2 in (("m", s2m), ("s", s2s)):
                    for n in range(NCH):
                        sl = slice(512 * n, 512 * (n + 1))
                        nc.scalar.activation(out=s2[:, sl],
                                             in_=psl2[br][n][:],
                                             func=AF.Copy)

                # ---------- layer-2 bmm + BN + relu; outputs ----------
                cat_off = {"m": 3 * C, "s": 3 * C + NP * C}
                bnf_off = {"m": C, "s": 2 * C}
                for br, s2 in (("m", s2m), ("s", s2s)):
                    x2 = mm.tile([R, C], BF, tag=f"x1{br}")
                    bmm_bn_relu(s2, s2r, t2r, x2, f"{br}2")
                    off = cat_off[br]
                    nc.scalar.dma_start(
                        out=out_p[:, off:off + NP * C].rearrange(
                            "b (p d) -> b p d", d=C),
                        in_=x2[:])
                    # mean over parts + BN(gn)
                    bnf = mm.tile([BL, C], BF, tag="bnf")
                    for n in range(NCH):
                        sl = slice(512 * n, 512 * (n + 1))
                        pf = ps.tile([BL, 512], F32, tag="ps",
                                     name=f"pf{br}{n}")
                        nc.tensor.matmul(pf[:], onesblk[:], x2[:, sl],
                                         start=True, stop=True)
                        nc.vector.tensor_tensor(bnf[:, sl], pf[:],
                                                sgn[:, sl], OP.mult)
                        nc.gpsimd.tensor_tensor(bnf[:, sl], bnf[:, sl],
                                                tgn[:, sl], OP.add)
                    boff = bnf_off[br]
                    nc.scalar.dma_start(out=out_p[:, boff:boff + C],
                                        in_=bnf[:])
                    if br == "m":
                        for b in range(2, 5):
                            gap_mm(b)

                # ---------- GAP finish + bnfeat_global (drains last) ------
                for b in range(5, BL):
                    gap_mm(b)
                Gn = cs.tile([BL, C], BF)
                for n in range(NCH):
                    sl = slice(512 * n, 512 * (n + 1))
                    nc.vector.tensor_tensor(Gn[:, sl], pgap[n][:],
                                            sgb[:, sl], OP.mult)
                    nc.gpsimd.tensor_tensor(Gn[:, sl], Gn[:, sl],
                                            tgb[:, sl], OP.add)
                nc.scalar.dma_start(out=out_p[:, 0:C], in_=Gn[:])

    legalize_waits(nc)
    return nc


_CACHE = {}


def _fold_bn(g, be, rm, rv, blin):
    s = g / np.sqrt(rv + EPS)
    t = (blin - rm) * s + be
    return s, t


def kernel(_run_kwargs=None, **inputs):
    run_kwargs = _run_kwargs or {}
    if "nc" not in _CACHE:
        _CACHE["nc"] = build_bass()
    nc = _CACHE["nc"]

    B = inputs["x_global"].shape[0]
    n_cores = 8
    bl = B // n_cores

    f = {k: np.asarray(inputs[k], np.float32) for k in inputs if k != "mask"}

    # BN folds (parameter preprocessing, replicated per core)
    s1, t1 = _fold_bn(f["g1"].reshape(NP, C), f["be1"].reshape(NP, C),
                      f["rm1"].reshape(NP, C), f["rv1"].reshape(NP, C),
                      f["b1"][None, :])
    s2, t2 = _fold_bn(f["g2"].reshape(NP, C), f["be2"].reshape(NP, C),
                      f["rm2"].reshape(NP, C), f["rv2"].reshape(NP, C),
                      f["b2"][None, :])
    reps = np.ascontiguousarray(np.stack(
        [np.tile(s1, (BL, 1)), np.tile(t1, (BL, 1)),
         np.tile(s2, (BL, 1)), np.tile(t2, (BL, 1))], axis=1)).astype(NPBF)
    sgb_ = f["gb_g"] / np.sqrt(f["gb_rv"] + EPS)
    tgb_ = f["gb_b"] - f["gb_rm"] * sgb_
    sgbt = np.ascontiguousarray(np.stack(
        [np.tile((sgb_ / HW)[None, :], (BL, 1)),
         np.tile(tgb_[None, :], (BL, 1))], axis=1)).astype(NPBF)
    sgn_ = f["gn_g"] / np.sqrt(f["gn_rv"] + EPS)
    tgn_ = f["gn_b"] - f["gn_rm"] * sgn_
    sgnt = np.ascontiguousarray(np.stack(
        [np.tile(sgn_[None, :], (BL, 1)),
         np.tile(tgn_[None, :], (BL, 1))], axis=1)).astype(NPBF)
    w1 = f["W1"].astype(NPBF)
    w2 = f["W2"].astype(NPBF)

    # block-diag strictly-lower-triangular (constant)
    l72 = np.zeros((R, R), np.float32)
    blk = np.tril(np.ones((NP, NP), np.float32), -1).T  # [q,r]=1 iff q<r
    for b in range(BL):
        l72[NP * b:NP * (b + 1), NP * b:NP * (b + 1)] = blk
    l72 = l72.astype(NPBF)

    mask_ds = np.asarray(inputs["mask"])[:, 0, ::16, ::16]  # (B, 16, 16)

    in_maps = []
    for c in range(n_cores):
        sl = slice(c * bl, (c + 1) * bl)
        # mask (bl,16,16) -> [p, h, b] with hw = 128*h + p
        md = mask_ds[sl].reshape(bl, 256).T.reshape(2, 128, bl)
        adj = f["adj"][sl]
        bd = np.zeros((R, R), np.float32)
        for b in range(bl):
            bd[NP * b:NP * (b + 1), NP * b:NP * (b + 1)] = adj[b].T
        bdl = np.ascontiguousarray(np.stack(
            [bd.astype(NPBF), l72], axis=1))
        m = {
            "xg": np.ascontiguousarray(f["x_global"][sl]).reshape(
                bl, C, HW).astype(NPBF),
            "xc": np.ascontiguousarray(f["x_gcn"][sl]).reshape(
                bl, C, HW).astype(NPBF),
            "mkp": np.ascontiguousarray(md.transpose(1, 0, 2)).astype(np.int32),
            "bdl": bdl,
            "W1": w1, "W2": w2,
            "reps": reps,
            "sgbt": sgbt, "sgnt": sgnt,
        }
        in_maps.append(m)

    from concourse.bass_utils import run_bass_kernel_spmd
    res = run_bass_kernel_spmd(nc, in_maps, list(range(n_cores)), **run_kwargs)
    out = np.concatenate(
        [np.asarray(res.results[c]["out"]).astype(np.float32)
         for c in range(n_cores)], axis=0)
    _CACHE["last_results"] = res
    return out
